# revision 19
# baseline (speedup 1.0000x reference)
"""Causal self-attention with rotary embeddings (B=2, T=2048, D=1024, H=16,
d_k=64) on 8 Trainium2 NeuronCores.

Sharding: core c handles batch b = c//4 and 4 heads (c%4)*4..+4 — data
parallel on B, tensor parallel on heads.  Each core computes its heads'
qkv projection, RoPE, causal attention, and a partial output projection
over its 256 attention channels; the host sums the 4 partials per batch.

Layout tricks:
  * all matmul operands are bf16 (fp32 on the PE costs ~2 cycles/row;
    bf16 is 1).  PSUM accumulation stays fp32.
  * q/k channels are de-interleaved host-side (RoPE pair -> half-split
    form) and packed 2 heads per 128-partition tile; scores matmuls are
    row-tiled K=64 pairs.
  * RoPE swap (+/- sign) is a 128x128 permutation matmul on TensorE; the
    cos/sin elementwise work runs on VectorE fused with PSUM eviction,
    and the qkv bias rides along as the per-partition scalar operand of
    scalar_tensor_tensor (v bias: broadcast add at PSUM eviction).
  * softmax skips max-subtraction (scores ~ N(0,1), bounded) and folds the
    denominator into attn@v as an extra ones-column of v; the divide is a
    per-head broadcast-reciprocal multiply at eviction.
  * causal masking is block-granular: scores/exp/attn@v matmuls under the
    block diagonal are truncated to the live query range.
"""

import sys

sys.path.insert(0, "/opt/trn_rl_repo")

import numpy as np
import ml_dtypes

import concourse.bacc as bacc
import concourse.tile as tile
from concourse import mybir
from concourse.bass_utils import run_bass_kernel_spmd

F32 = mybir.dt.float32
BF16 = mybir.dt.bfloat16

B, T, D = 2, 2048, 1024
NH, DK = 16, 64
THETA = 10000.0
NCORES = 8
HEADS_PER_CORE = 4

TC512 = T // 512        # 4   i-chunks of 512
TC128 = T // 128        # 16  t/j-chunks of 128
KC = D // 128           # 8   d_model contraction chunks

ADD = mybir.AluOpType.add
MUL = mybir.AluOpType.mult


def build_program(debug=False):
    nc = bacc.Bacc("TRN2", target_bir_lowering=False, debug=False)

    # all big operands arrive pre-packed in SBUF layout (partition-major,
    # fully contiguous per partition) so DMA moves 4-8KB bursts instead of
    # 1KB strided runs.  XT is additionally 512-token-block-major.
    XT = nc.dram_tensor("XT", [128, TC512 * KC * 512], BF16, kind="ExternalInput").ap()
    WQK = nc.dram_tensor("WQK", [128, KC * 512], BF16, kind="ExternalInput").ap()
    WV = nc.dram_tensor("WV", [128, KC * 256], BF16, kind="ExternalInput").ap()
    WOUT = nc.dram_tensor("WOUT", [128, 2 * D], BF16, kind="ExternalInput").ap()
    PSW = nc.dram_tensor("PSW", [128, 128], BF16, kind="ExternalInput").ap()
    CQ = nc.dram_tensor("CQ", [128, T], BF16, kind="ExternalInput").ap()
    SQ = nc.dram_tensor("SQ", [128, T], BF16, kind="ExternalInput").ap()
    BQK = nc.dram_tensor("BQK", [128, 4], F32, kind="ExternalInput").ap()
    BV = nc.dram_tensor("BV", [1, 256], F32, kind="ExternalInput").ap()
    TRI = nc.dram_tensor("TRI", [128, 128], BF16, kind="ExternalInput").ap()
    OUT = nc.dram_tensor("OUT", [T, D], BF16, kind="ExternalOutput").ap()
    if debug:
        DBG_QKT = nc.dram_tensor("DBG_QKT", [128, 4 * T], BF16, kind="ExternalOutput").ap()
        DBG_V = nc.dram_tensor("DBG_V", [128, TC128 * 260], BF16, kind="ExternalOutput").ap()
        DBG_ATT = nc.dram_tensor("DBG_ATT", [128, 2 * T], BF16, kind="ExternalOutput").ap()

    EXP = mybir.ActivationFunctionType.Exp

    with tile.TileContext(nc) as tc:
        with (
            tc.tile_pool(name="persist", bufs=1) as persist,
            tc.tile_pool(name="p1w", bufs=1) as p1w,
            tc.tile_pool(name="p1t", bufs=3) as p1t,
            tc.tile_pool(name="p2e", bufs=4) as p2e,
            tc.tile_pool(name="p2bc", bufs=1) as p2bc,
            tc.tile_pool(name="p2r", bufs=1) as p2r,
            tc.tile_pool(name="pj", bufs=2, space="PSUM") as pj,
            tc.tile_pool(name="sps", bufs=2, space="PSUM") as sps,
            tc.tile_pool(name="avps", bufs=2, space="PSUM") as avps,
        ):
            # ---- persistent tiles --------------------------------------
            qkT = persist.tile([128, 4 * T], BF16, tag="qkT")       # Qp0 Kp0 Qp1 Kp1
            v_sb = persist.tile([128, TC128 * 260], BF16, tag="v_sb")  # [jc, head, 64+1]
            attnT = persist.tile([128, 2 * T], BF16, tag="attnT")   # c-chunks x t
            wout_sb = persist.tile([128, 2 * D], BF16, tag="wout_sb")
            tri_sb = persist.tile([128, 128], BF16, tag="tri_sb")

            x_sb = p1w.tile([128, KC * T], BF16, tag="x_sb")
            wqk_sb = p1w.tile([128, KC * 512], BF16, tag="wqk_sb")
            wv_sb = p1w.tile([128, KC * 256], BF16, tag="wv_sb")
            psw_sb = p1w.tile([128, 128], BF16, tag="psw_sb")
            cq_sb = p1w.tile([128, T], BF16, tag="cq_sb")
            sq_sb = p1w.tile([128, T], BF16, tag="sq_sb")
            bqk_sb = p1w.tile([128, 4], F32, tag="bqk_sb")
            bv_sb = p1w.tile([1, 256], F32, tag="bv_sb")
            bvbc_sb = p1w.tile([128, 256], F32, tag="bvbc_sb")

            # x column layout: block-major (n*4096 + k*512 + t), identical
            # in DRAM and SBUF, so every load is a fully-contiguous burst.
            def xcol(k, n512, t0):
                return n512 * (KC * 512) + k * 512 + t0

            def load_x_block(n):
                b0 = n * (KC * 512)
                nc.sync.dma_start(x_sb[:, b0:b0 + 2048], XT[:, b0:b0 + 2048])
                nc.sync.dma_start(x_sb[:, b0 + 2048:b0 + 4096], XT[:, b0 + 2048:b0 + 4096])

            # preamble: x block 0 on gpsimd, wqk on sync (k ascending so
            # the accumulation matmuls start as pieces land); small tables
            # on scalar behind the tiny psw/bias loads.
            nc.gpsimd.dma_start(x_sb[:, 0:1024], XT[:, 0:1024])
            nc.sync.dma_start(wqk_sb[:, 0:1024], WQK[:, 0:1024])
            nc.gpsimd.dma_start(x_sb[:, 1024:2048], XT[:, 1024:2048])
            nc.sync.dma_start(wqk_sb[:, 1024:2048], WQK[:, 1024:2048])
            nc.gpsimd.dma_start(x_sb[:, 2048:3072], XT[:, 2048:3072])
            nc.sync.dma_start(wqk_sb[:, 2048:3072], WQK[:, 2048:3072])
            nc.gpsimd.dma_start(x_sb[:, 3072:4096], XT[:, 3072:4096])
            nc.sync.dma_start(wqk_sb[:, 3072:4096], WQK[:, 3072:4096])
            nc.scalar.dma_start(psw_sb[:], PSW[:])
            nc.scalar.dma_start(bqk_sb[:], BQK[:])
            nc.scalar.dma_start(bv_sb[:], BV[:])
            nc.scalar.dma_start(cq_sb[:], CQ[:])
            nc.scalar.dma_start(sq_sb[:], SQ[:])
            nc.scalar.dma_start(tri_sb[:], TRI[:])
            nc.gpsimd.dma_start(wv_sb[:], WV[:])
            nc.gpsimd.partition_broadcast(bvbc_sb[:], bv_sb[:], channels=128)

            # ones columns of v_aug: one strided memset
            v4 = v_sb[:].rearrange("p (jc h e) -> p jc h e", jc=TC128, h=4)
            nc.vector.memset(v4[:, :, :, 64:65], 1.0)

            # ---------------- building blocks ---------------------------
            def qk_proj_chunk(m, n):
                """project q/k m-chunk (128 channels) for t-chunk n (512),
                apply bias + rope during eviction."""
                nsl = slice(n * 512, (n + 1) * 512)
                ps = pj.tile([128, 512], F32, tag="pj", name=f"psqk_{m}_{n}")
                for k in range(KC):
                    nc.tensor.matmul(
                        ps[:],
                        wqk_sb[:, k * 512 + m * 128:k * 512 + (m + 1) * 128],
                        x_sb[:, xcol(k, n, 0):xcol(k, n, 512)],
                        start=(k == 0), stop=(k == KC - 1),
                    )
                tmp_s = p1t.tile([128, 512], BF16, tag="tmp_s", name=f"tmps_{m}_{n}")
                tmp_c = p1t.tile([128, 512], BF16, tag="tmp_c", name=f"tmpc_{m}_{n}")
                bcol = bqk_sb[:, m:m + 1]
                nc.vector.scalar_tensor_tensor(tmp_s[:], ps[:], bcol, sq_sb[:, nsl], ADD, MUL)
                nc.vector.scalar_tensor_tensor(tmp_c[:], ps[:], bcol, cq_sb[:, nsl], ADD, MUL)
                sw = pj.tile([128, 512], F32, tag="pj", name=f"sw_{m}_{n}")
                nc.tensor.matmul(sw[:], psw_sb[:], tmp_s[:], start=True, stop=True)
                nc.vector.tensor_add(qkT[:, m * T + n * 512:m * T + (n + 1) * 512], sw[:], tmp_c[:])

            def v_proj_chunk(tcc):
                tsl = slice(tcc * 128, (tcc + 1) * 128)
                psv = pj.tile([128, 256], F32, tag="pj", name=f"psv_{tcc}")
                for k in range(KC):
                    t0 = (tcc % 4) * 128
                    nc.tensor.matmul(
                        psv[:],
                        x_sb[:, xcol(k, tcc // 4, t0):xcol(k, tcc // 4, t0 + 128)],
                        wv_sb[:, k * 256:(k + 1) * 256],
                        start=(k == 0), stop=(k == KC - 1),
                    )
                vdst = v_sb[:, tcc * 260:(tcc + 1) * 260].rearrange(
                    "p (h e) -> p h e", h=4)[:, :, 0:64]
                vsrc = psv[:].rearrange("p (h e) -> p h e", e=64)
                bv4 = bvbc_sb[:].rearrange("p (h e) -> p h e", e=64)
                nc.vector.tensor_add(vdst, vsrc, bv4)

            def attn_ic(p, ic, fillers=(), pre=None):
                """attention for head-pair p, query chunk ic (512 queries).
                fillers: callables paced evenly across the jc loop (PE
                density while the exp stream gates).  pre: {jc: [fns]}
                forced placements for order-critical work (emitted before
                that jc's attention instructions)."""
                fillers = list(fillers)
                pre = dict(pre or {})
                total = len(fillers)
                done = 0
                qof = (2 * p) * T
                kof = (2 * p + 1) * T
                njc = 4 * ic + 4
                av = [avps.tile([65, 512], F32, tag="av", name=f"av_{p}_{ic}_{i}") for i in range(2)]
                for jc in range(njc):
                    for fn in pre.pop(jc, ()):
                        fn()
                    rel = jc - 4 * ic
                    ls = 0 if rel < 0 else rel * 128
                    e_pair = p2e.tile([128, 1024], BF16, tag="e_t", name=f"e_{p}_{ic}_{jc}")
                    s_pair = sps.tile([128, 1024], F32, tag="s_ps", name=f"s_{p}_{ic}_{jc}")
                    for hh in range(2):
                        pof = hh * 64
                        nc.tensor.matmul(
                            s_pair[:, hh * 512 + ls:(hh + 1) * 512],
                            qkT[pof:pof + 64, kof + jc * 128:kof + (jc + 1) * 128],
                            qkT[pof:pof + 64, qof + ic * 512 + ls:qof + (ic + 1) * 512],
                            start=True, stop=True,
                        )
                    sv = s_pair[:].rearrange("p (h w) -> p h w", h=2)
                    ev = e_pair[:].rearrange("p (h w) -> p h w", h=2)
                    nc.scalar.activation(ev[:, :, ls:512], sv[:, :, ls:512], EXP, scale=8.0)
                    if rel >= 0:
                        tsl_ = slice(rel * 128, (rel + 1) * 128)
                        nc.vector.tensor_mul(ev[:, :, tsl_], ev[:, :, tsl_],
                                             tri_sb[:].unsqueeze(1).broadcast_to([128, 2, 128]))
                    for hh in range(2):
                        nc.tensor.matmul(
                            av[hh][:, ls:512],
                            v_sb[:, jc * 260 + (2 * p + hh) * 65:jc * 260 + (2 * p + hh) * 65 + 65],
                            e_pair[:, hh * 512 + ls:(hh + 1) * 512],
                            start=(jc == 0), stop=(jc == njc - 1),
                            skip_group_check=True,
                        )
                    while done < total and (total - done) > (njc - 1 - jc):
                        fillers[done]()
                        done += 1
                for hh in range(2):
                    head = 2 * p + hh
                    den = p2r.tile([1, 512], F32, tag="den", name=f"den_{p}_{ic}_{hh}")
                    nc.vector.tensor_copy(den[:], av[hh][64:65, :])
                    rec = p2r.tile([1, 512], F32, tag="rec", name=f"rec_{p}_{ic}_{hh}")
                    nc.vector.reciprocal_approx_fast(rec[:], den[:])
                    bc_sb = p2bc.tile([64, 512], F32, tag="bc_sb", name=f"bc_{p}_{ic}_{hh}")
                    nc.gpsimd.partition_broadcast(bc_sb[:], rec[:], channels=64)
                    cof = (head // 2) * T
                    pof = (head % 2) * 64
                    nc.vector.tensor_mul(
                        attnT[pof:pof + 64, cof + ic * 512:cof + (ic + 1) * 512],
                        av[hh][0:64, :], bc_sb[:],
                    )

            def out_proj_chunk(tcc):
                tsl = slice(tcc * 128, (tcc + 1) * 128)
                po_sb = p1t.tile([128, 1024], BF16, tag="po_sb", name=f"po_sb_{tcc}")
                for oc in range(2):
                    po = pj.tile([128, 512], F32, tag="pj", name=f"po_{tcc}_{oc}")
                    for cc in range(2):
                        nc.tensor.matmul(
                            po[:],
                            attnT[:, cc * T + tcc * 128:cc * T + (tcc + 1) * 128],
                            wout_sb[:, cc * D + oc * 512:cc * D + (oc + 1) * 512],
                            start=(cc == 0), stop=(cc == 1),
                        )
                    osl = slice(oc * 512, (oc + 1) * 512)
                    if oc == 0:
                        nc.vector.tensor_copy(po_sb[:, osl], po[:])
                    else:
                        nc.scalar.copy(po_sb[:, osl], po[:])
                nc.sync.dma_start(OUT[tsl, :], po_sb[:])

            # ---------------- schedule: n-major waves -------------------
            # wave n: project all qk m-chunks + v chunks for t-block n, run
            # both pairs' attention for query block n, and the out
            # projection for t-chunks completed in wave n-1.
            load_x_block(1)
            for m in range(4):
                qk_proj_chunk(m, 0)
            for tcc in range(4):
                v_proj_chunk(tcc)
            for n in range(TC512):
                fill_a, fill_b = [], []
                if n < 3:
                    nx = n + 1
                    fill0 = []
                    if nx + 1 < TC512:
                        fill0 += [lambda b=nx + 1: load_x_block(b)]
                    if nx < 3:
                        fill0 += [(lambda m=m: qk_proj_chunk(m, nx)) for m in range(4)]
                    else:
                        # block 3: q chunks projected here; the k chunks are
                        # deferred into wave 3 (keys 12-15 are only consumed
                        # from jc=12, and wave 3 is otherwise exp-bound)
                        fill0 += [(lambda m=m: qk_proj_chunk(m, nx)) for m in (0, 2)]
                    fill0 += [(lambda t=t: v_proj_chunk(t)) for t in range(4 * nx, 4 * nx + 4)]
                    if n == 0:
                        def load_wout():
                            nc.sync.dma_start(wout_sb[:], WOUT[:])
                        fill0 += [load_wout]
                    half = (len(fill0) + 1) // 2
                    fill_a += fill0[:half]
                    fill_b += fill0[half:]
                pre_a = pre_b = None
                if n == 3:
                    # deferred block-3 k projections: keys 12-15 are only
                    # consumed from jc=12, but must be written well before —
                    # force them at the head of each pair's jc loop
                    pre_a = {0: [lambda: qk_proj_chunk(1, 3)]}
                    pre_b = {0: [lambda: qk_proj_chunk(3, 3)]}
                op_sched = {1: range(0, 4), 2: range(4, 8), 3: range(8, 12)}
                if n in op_sched:
                    ops = [(lambda t=t: out_proj_chunk(t)) for t in op_sched[n]]
                    fill_a += ops[:2]
                    fill_b += ops[2:]
                attn_ic(0, n, fill_a, pre=pre_a)
                attn_ic(1, n, fill_b, pre=pre_b)
            for tcc in range(12, 16):
                out_proj_chunk(tcc)

            if debug:
                nc.sync.dma_start(DBG_QKT[:], qkT[:])
                nc.sync.dma_start(DBG_V[:], v_sb[:])
                nc.sync.dma_start(DBG_ATT[:], attnT[:])

    nc.compile()
    return nc


_DEINT = list(range(0, DK, 2)) + list(range(1, DK, 2))


def _rope_tables():
    j = np.arange(DK // 2, dtype=np.float64)
    inv_freq = THETA ** (-2.0 * j / DK)
    t = np.arange(T, dtype=np.float64)
    ang = t[None, :] * inv_freq[:, None]          # [32, T]
    ang = np.tile(ang, (4, 1))                    # [128, T]
    return np.cos(ang).astype(np.float32), np.sin(ang).astype(np.float32)


def _psw():
    M = np.zeros((128, 128), dtype=np.float32)
    for p in range(128):
        pm = p % 64
        if pm < 32:
            M[p, p + 32] = -1.0
        else:
            M[p, p - 32] = 1.0
    return np.ascontiguousarray(M.T)


def shard_inputs(x, Wqkv, bqkv, Wout, bout):
    bf16 = ml_dtypes.bfloat16
    x = np.asarray(x, dtype=np.float32)
    Wqkv = np.asarray(Wqkv, dtype=np.float32)
    bqkv = np.asarray(bqkv, dtype=np.float32)
    Wout = np.asarray(Wout, dtype=np.float32)

    cos_t, sin_t = _rope_tables()
    cq = np.ascontiguousarray(cos_t / 8.0).astype(bf16)
    sq = np.ascontiguousarray(sin_t / 8.0).astype(bf16)
    psw = _psw().astype(bf16)
    tri = np.triu(np.ones((128, 128), dtype=np.float32)).astype(bf16)

    def pack(w_t, kc):
        # [kc*128, cols] channel-major -> [128, kc*cols] SBUF-contiguous
        cols = w_t.shape[1]
        return np.ascontiguousarray(
            w_t.reshape(kc, 128, cols).transpose(1, 0, 2).reshape(128, kc * cols)
        ).astype(bf16)

    xt = {}
    for b in range(B):
        # [p, n*4096 + k*512 + t] block-major SBUF layout
        xt[b] = np.ascontiguousarray(
            x[b].T.reshape(KC, 128, TC512, 512).transpose(1, 2, 0, 3).reshape(128, -1)
        ).astype(bf16)

    in_maps = []
    for c in range(NCORES):
        b = c // 4
        heads = [4 * (c % 4) + i for i in range(HEADS_PER_CORE)]
        # chunk order: [Qp0 | Kp0 | Qp1 | Kp1], each 128 rows (2 heads x 64)
        qk_rows = []
        for p in range(2):
            qrows, krows = [], []
            for h in (2 * p, 2 * p + 1):
                H = heads[h]
                qrows += [H * 192 + j for j in _DEINT]
                krows += [H * 192 + 64 + j for j in _DEINT]
            qk_rows += qrows + krows
        v_rows = []
        for h in range(4):
            H = heads[h]
            v_rows += [H * 192 + 128 + j for j in range(DK)]
        vch_out = []
        for h in range(4):
            H = heads[h]
            vch_out += [H * 64 + j for j in range(DK)]

        in_maps.append({
            "XT": xt[b],
            "WQK": pack(Wqkv[qk_rows].T, KC),
            "WV": pack(Wqkv[v_rows].T, KC),
            "WOUT": pack(Wout[:, vch_out].T, 2),
            "PSW": psw,
            "CQ": cq,
            "SQ": sq,
            "BQK": np.ascontiguousarray(bqkv[qk_rows].reshape(4, 128).T.astype(np.float32)),
            "BV": np.ascontiguousarray(bqkv[v_rows].reshape(1, 256).astype(np.float32)),
            "TRI": tri,
        })
    return in_maps


_CACHED = {}


def _get_program(debug=False):
    key = bool(debug)
    if key not in _CACHED:
        _CACHED[key] = build_program(debug=debug)
    return _CACHED[key]


def run_cores(inputs, debug=False, trace=False, tmpdir=None):
    nc = _get_program(debug=debug)
    in_maps = shard_inputs(**inputs)
    res = run_bass_kernel_spmd(
        nc, in_maps, core_ids=list(range(NCORES)), trace=trace, tmpdir=tmpdir,
    )
    return res


def combine(results, bout):
    bout = np.asarray(bout, dtype=np.float32)
    out = np.empty((B, T, D), dtype=np.float32)
    for b in range(B):
        acc = results[4 * b]["OUT"].astype(np.float32)
        for c in range(4 * b + 1, 4 * b + 4):
            acc += results[c]["OUT"].astype(np.float32)
        out[b] = acc + bout[None, :]
    return out


def kernel(x, Wqkv, bqkv, Wout, bout):
    res = run_cores(dict(x=x, Wqkv=Wqkv, bqkv=bqkv, Wout=Wout, bout=bout))
    return combine(res.results, bout)


# revision 20
# speedup vs baseline: 1.0037x; 1.0037x over previous
"""Causal self-attention with rotary embeddings (B=2, T=2048, D=1024, H=16,
d_k=64) on 8 Trainium2 NeuronCores.

Sharding: core c handles batch b = c//4 and 4 heads (c%4)*4..+4 — data
parallel on B, tensor parallel on heads.  Each core computes its heads'
qkv projection, RoPE, causal attention, and a partial output projection
over its 256 attention channels; the host sums the 4 partials per batch.

Layout tricks:
  * all matmul operands are bf16 (fp32 on the PE costs ~2 cycles/row;
    bf16 is 1).  PSUM accumulation stays fp32.
  * q/k channels are de-interleaved host-side (RoPE pair -> half-split
    form) and packed 2 heads per 128-partition tile; scores matmuls are
    row-tiled K=64 pairs.
  * RoPE swap (+/- sign) is a 128x128 permutation matmul on TensorE; the
    cos/sin elementwise work runs on VectorE fused with PSUM eviction,
    and the qkv bias rides along as the per-partition scalar operand of
    scalar_tensor_tensor (v bias: broadcast add at PSUM eviction).
  * softmax skips max-subtraction (scores ~ N(0,1), bounded) and folds the
    denominator into attn@v as an extra ones-column of v; the divide is a
    per-head broadcast-reciprocal multiply at eviction.
  * causal masking is block-granular: scores/exp/attn@v matmuls under the
    block diagonal are truncated to the live query range.
"""

import sys

sys.path.insert(0, "/opt/trn_rl_repo")

import numpy as np
import ml_dtypes

import concourse.bacc as bacc
import concourse.tile as tile
from concourse import mybir
from concourse.bass_utils import run_bass_kernel_spmd

F32 = mybir.dt.float32
BF16 = mybir.dt.bfloat16

B, T, D = 2, 2048, 1024
NH, DK = 16, 64
THETA = 10000.0
NCORES = 8
HEADS_PER_CORE = 4

TC512 = T // 512        # 4   i-chunks of 512
TC128 = T // 128        # 16  t/j-chunks of 128
KC = D // 128           # 8   d_model contraction chunks

ADD = mybir.AluOpType.add
MUL = mybir.AluOpType.mult


def build_program(debug=False):
    nc = bacc.Bacc("TRN2", target_bir_lowering=False, debug=False)

    # all big operands arrive pre-packed in SBUF layout (partition-major,
    # fully contiguous per partition) so DMA moves 4-8KB bursts instead of
    # 1KB strided runs.  XT is additionally 512-token-block-major.
    XT = nc.dram_tensor("XT", [128, TC512 * KC * 512], BF16, kind="ExternalInput").ap()
    WQK = nc.dram_tensor("WQK", [128, KC * 512], BF16, kind="ExternalInput").ap()
    WV = nc.dram_tensor("WV", [128, KC * 256], BF16, kind="ExternalInput").ap()
    WOUT = nc.dram_tensor("WOUT", [128, 2 * D], BF16, kind="ExternalInput").ap()
    PSW = nc.dram_tensor("PSW", [128, 128], BF16, kind="ExternalInput").ap()
    CQ = nc.dram_tensor("CQ", [128, T], BF16, kind="ExternalInput").ap()
    SQ = nc.dram_tensor("SQ", [128, T], BF16, kind="ExternalInput").ap()
    BQK = nc.dram_tensor("BQK", [128, 4], F32, kind="ExternalInput").ap()
    BV = nc.dram_tensor("BV", [1, 256], F32, kind="ExternalInput").ap()
    TRI = nc.dram_tensor("TRI", [128, 128], BF16, kind="ExternalInput").ap()
    OUT = nc.dram_tensor("OUT", [T, D], BF16, kind="ExternalOutput").ap()
    if debug:
        DBG_QKT = nc.dram_tensor("DBG_QKT", [128, 4 * T], BF16, kind="ExternalOutput").ap()
        DBG_V = nc.dram_tensor("DBG_V", [128, TC128 * 260], BF16, kind="ExternalOutput").ap()
        DBG_ATT = nc.dram_tensor("DBG_ATT", [128, 2 * T], BF16, kind="ExternalOutput").ap()

    EXP = mybir.ActivationFunctionType.Exp

    with tile.TileContext(nc) as tc:
        with (
            tc.tile_pool(name="persist", bufs=1) as persist,
            tc.tile_pool(name="p1w", bufs=1) as p1w,
            tc.tile_pool(name="p1t", bufs=3) as p1t,
            tc.tile_pool(name="p2e", bufs=4) as p2e,
            tc.tile_pool(name="p2bc", bufs=2) as p2bc,
            tc.tile_pool(name="p2r", bufs=2) as p2r,
            tc.tile_pool(name="pj", bufs=2, space="PSUM") as pj,
            tc.tile_pool(name="sps", bufs=2, space="PSUM") as sps,
            tc.tile_pool(name="avps", bufs=2, space="PSUM") as avps,
        ):
            # ---- persistent tiles --------------------------------------
            qkT = persist.tile([128, 4 * T], BF16, tag="qkT")       # Qp0 Kp0 Qp1 Kp1
            v_sb = persist.tile([128, TC128 * 260], BF16, tag="v_sb")  # [jc, head, 64+1]
            attnT = persist.tile([128, 2 * T], BF16, tag="attnT")   # c-chunks x t
            wout_sb = persist.tile([128, 2 * D], BF16, tag="wout_sb")
            tri_sb = persist.tile([128, 128], BF16, tag="tri_sb")

            x_sb = p1w.tile([128, KC * T], BF16, tag="x_sb")
            wqk_sb = p1w.tile([128, KC * 512], BF16, tag="wqk_sb")
            wv_sb = p1w.tile([128, KC * 256], BF16, tag="wv_sb")
            psw_sb = p1w.tile([128, 128], BF16, tag="psw_sb")
            cq_sb = p1w.tile([128, T], BF16, tag="cq_sb")
            sq_sb = p1w.tile([128, T], BF16, tag="sq_sb")
            bqk_sb = p1w.tile([128, 4], F32, tag="bqk_sb")
            bv_sb = p1w.tile([1, 256], F32, tag="bv_sb")
            bvbc_sb = p1w.tile([128, 256], F32, tag="bvbc_sb")

            # x column layout: block-major (n*4096 + k*512 + t), identical
            # in DRAM and SBUF, so every load is a fully-contiguous burst.
            def xcol(k, n512, t0):
                return n512 * (KC * 512) + k * 512 + t0

            def load_x_block(n):
                b0 = n * (KC * 512)
                nc.sync.dma_start(x_sb[:, b0:b0 + 2048], XT[:, b0:b0 + 2048])
                nc.sync.dma_start(x_sb[:, b0 + 2048:b0 + 4096], XT[:, b0 + 2048:b0 + 4096])

            # preamble: x block 0 on gpsimd, wqk on sync (k ascending so
            # the accumulation matmuls start as pieces land); small tables
            # on scalar behind the tiny psw/bias loads.
            nc.gpsimd.dma_start(x_sb[:, 0:1024], XT[:, 0:1024])
            nc.sync.dma_start(wqk_sb[:, 0:1024], WQK[:, 0:1024])
            nc.gpsimd.dma_start(x_sb[:, 1024:2048], XT[:, 1024:2048])
            nc.sync.dma_start(wqk_sb[:, 1024:2048], WQK[:, 1024:2048])
            nc.gpsimd.dma_start(x_sb[:, 2048:3072], XT[:, 2048:3072])
            nc.sync.dma_start(wqk_sb[:, 2048:3072], WQK[:, 2048:3072])
            nc.gpsimd.dma_start(x_sb[:, 3072:4096], XT[:, 3072:4096])
            nc.sync.dma_start(wqk_sb[:, 3072:4096], WQK[:, 3072:4096])
            nc.scalar.dma_start(psw_sb[:], PSW[:])
            nc.scalar.dma_start(bqk_sb[:], BQK[:])
            nc.scalar.dma_start(bv_sb[:], BV[:])
            nc.scalar.dma_start(cq_sb[:], CQ[:])
            nc.scalar.dma_start(sq_sb[:], SQ[:])
            nc.scalar.dma_start(tri_sb[:], TRI[:])
            nc.gpsimd.dma_start(wv_sb[:], WV[:])
            nc.gpsimd.partition_broadcast(bvbc_sb[:], bv_sb[:], channels=128)

            # ones columns of v_aug: one strided memset
            v4 = v_sb[:].rearrange("p (jc h e) -> p jc h e", jc=TC128, h=4)
            nc.vector.memset(v4[:, :, :, 64:65], 1.0)

            # ---------------- building blocks ---------------------------
            def qk_proj_chunk(m, n):
                """project q/k m-chunk (128 channels) for t-chunk n (512),
                apply bias + rope during eviction."""
                nsl = slice(n * 512, (n + 1) * 512)
                ps = pj.tile([128, 512], F32, tag="pj", name=f"psqk_{m}_{n}")
                for k in range(KC):
                    nc.tensor.matmul(
                        ps[:],
                        wqk_sb[:, k * 512 + m * 128:k * 512 + (m + 1) * 128],
                        x_sb[:, xcol(k, n, 0):xcol(k, n, 512)],
                        start=(k == 0), stop=(k == KC - 1),
                    )
                tmp_s = p1t.tile([128, 512], BF16, tag="tmp_s", name=f"tmps_{m}_{n}")
                tmp_c = p1t.tile([128, 512], BF16, tag="tmp_c", name=f"tmpc_{m}_{n}")
                bcol = bqk_sb[:, m:m + 1]
                nc.vector.scalar_tensor_tensor(tmp_s[:], ps[:], bcol, sq_sb[:, nsl], ADD, MUL)
                nc.vector.scalar_tensor_tensor(tmp_c[:], ps[:], bcol, cq_sb[:, nsl], ADD, MUL)
                sw = pj.tile([128, 512], F32, tag="pj", name=f"sw_{m}_{n}")
                nc.tensor.matmul(sw[:], psw_sb[:], tmp_s[:], start=True, stop=True)
                nc.vector.tensor_add(qkT[:, m * T + n * 512:m * T + (n + 1) * 512], sw[:], tmp_c[:])

            def v_proj_chunk(tcc):
                tsl = slice(tcc * 128, (tcc + 1) * 128)
                psv = pj.tile([128, 256], F32, tag="pj", name=f"psv_{tcc}")
                for k in range(KC):
                    t0 = (tcc % 4) * 128
                    nc.tensor.matmul(
                        psv[:],
                        x_sb[:, xcol(k, tcc // 4, t0):xcol(k, tcc // 4, t0 + 128)],
                        wv_sb[:, k * 256:(k + 1) * 256],
                        start=(k == 0), stop=(k == KC - 1),
                    )
                vdst = v_sb[:, tcc * 260:(tcc + 1) * 260].rearrange(
                    "p (h e) -> p h e", h=4)[:, :, 0:64]
                vsrc = psv[:].rearrange("p (h e) -> p h e", e=64)
                bv4 = bvbc_sb[:].rearrange("p (h e) -> p h e", e=64)
                nc.vector.tensor_add(vdst, vsrc, bv4)

            def evict_heads(p, ic, av, q0, q1):
                """normalize av[q0:q1) into attnT (den reciprocal broadcast)."""
                w = q1 - q0
                for hh in range(2):
                    head = 2 * p + hh
                    den = p2r.tile([1, 512], F32, tag="den", name=f"den_{p}_{ic}_{hh}_{q0}")
                    nc.vector.tensor_copy(den[:, 0:w], av[hh][64:65, q0:q1])
                    rec = p2r.tile([1, 512], F32, tag="rec", name=f"rec_{p}_{ic}_{hh}_{q0}")
                    nc.vector.reciprocal_approx_fast(rec[:, 0:w], den[:, 0:w])
                    bc_sb = p2bc.tile([64, 512], F32, tag="bc_sb", name=f"bc_{p}_{ic}_{hh}_{q0}")
                    nc.gpsimd.partition_broadcast(bc_sb[:, 0:w], rec[:, 0:w], channels=64)
                    cof = (head // 2) * T
                    pof = (head % 2) * 64
                    nc.vector.tensor_mul(
                        attnT[pof:pof + 64, cof + ic * 512 + q0:cof + ic * 512 + q1],
                        av[hh][0:64, q0:q1], bc_sb[:, 0:w],
                    )

            def attn_ic(p, ic, fillers=(), even_fillers=(), pre=None):
                """attention for head-pair p, query chunk ic (512 queries).
                fillers: DMA-gated work, paced late (emitted only when the
                remaining jc slots require it).  even_fillers: compute-ready
                work paced evenly across the jc loop.  pre: {jc: [fns]}
                forced placements.  Queries [0:256) are evicted early at
                jc=njc-3 (their causal blocks are complete), shortening the
                tail chain and freeing av for the next attn_ic sooner."""
                fillers = list(fillers)
                even = list(even_fillers)
                pre = dict(pre or {})
                total, done = len(fillers), 0
                etotal, edone = len(even), 0
                qof = (2 * p) * T
                kof = (2 * p + 1) * T
                njc = 4 * ic + 4
                av = [avps.tile([65, 512], F32, tag="av", name=f"av_{p}_{ic}_{i}") for i in range(2)]
                for jc in range(njc):
                    for fn in pre.pop(jc, ()):
                        fn()
                    rel = jc - 4 * ic
                    ls = 0 if rel < 0 else rel * 128
                    e_pair = p2e.tile([128, 1024], BF16, tag="e_t", name=f"e_{p}_{ic}_{jc}")
                    s_pair = sps.tile([128, 1024], F32, tag="s_ps", name=f"s_{p}_{ic}_{jc}")
                    for hh in range(2):
                        pof = hh * 64
                        nc.tensor.matmul(
                            s_pair[:, hh * 512 + ls:(hh + 1) * 512],
                            qkT[pof:pof + 64, kof + jc * 128:kof + (jc + 1) * 128],
                            qkT[pof:pof + 64, qof + ic * 512 + ls:qof + (ic + 1) * 512],
                            start=True, stop=True,
                        )
                    sv = s_pair[:].rearrange("p (h w) -> p h w", h=2)
                    ev = e_pair[:].rearrange("p (h w) -> p h w", h=2)
                    nc.scalar.activation(ev[:, :, ls:512], sv[:, :, ls:512], EXP, scale=8.0)
                    if rel >= 0:
                        tsl_ = slice(rel * 128, (rel + 1) * 128)
                        nc.vector.tensor_mul(ev[:, :, tsl_], ev[:, :, tsl_],
                                             tri_sb[:].unsqueeze(1).broadcast_to([128, 2, 128]))
                    for hh in range(2):
                        nc.tensor.matmul(
                            av[hh][:, ls:512],
                            v_sb[:, jc * 260 + (2 * p + hh) * 65:jc * 260 + (2 * p + hh) * 65 + 65],
                            e_pair[:, hh * 512 + ls:(hh + 1) * 512],
                            start=(jc == 0), stop=(jc == njc - 1),
                            skip_group_check=True,
                        )
                    if jc == njc - 3:
                        evict_heads(p, ic, av, 0, 256)
                    ewant = -(-etotal * (jc + 1) // njc)
                    while edone < ewant:
                        even[edone]()
                        edone += 1
                    while done < total and (total - done) > (njc - 1 - jc):
                        fillers[done]()
                        done += 1
                evict_heads(p, ic, av, 256, 512)

            def out_proj_chunk(tcc):
                tsl = slice(tcc * 128, (tcc + 1) * 128)
                po_sb = p1t.tile([128, 1024], BF16, tag="po_sb", name=f"po_sb_{tcc}")
                for oc in range(2):
                    po = pj.tile([128, 512], F32, tag="pj", name=f"po_{tcc}_{oc}")
                    for cc in range(2):
                        nc.tensor.matmul(
                            po[:],
                            attnT[:, cc * T + tcc * 128:cc * T + (tcc + 1) * 128],
                            wout_sb[:, cc * D + oc * 512:cc * D + (oc + 1) * 512],
                            start=(cc == 0), stop=(cc == 1),
                        )
                    osl = slice(oc * 512, (oc + 1) * 512)
                    if oc == 0:
                        nc.vector.tensor_copy(po_sb[:, osl], po[:])
                    else:
                        nc.scalar.copy(po_sb[:, osl], po[:])
                nc.sync.dma_start(OUT[tsl, :], po_sb[:])

            # ---------------- schedule: n-major waves -------------------
            # wave n: project all qk m-chunks + v chunks for t-block n, run
            # both pairs' attention for query block n, and the out
            # projection for t-chunks completed in wave n-1.
            load_x_block(1)
            for m in range(4):
                qk_proj_chunk(m, 0)
            for tcc in range(4):
                v_proj_chunk(tcc)
            for n in range(TC512):
                fill_a, fill_b = [], []
                if n < 3:
                    nx = n + 1
                    fill0 = []
                    if nx + 1 < TC512:
                        fill0 += [lambda b=nx + 1: load_x_block(b)]
                    if nx < 3:
                        fill0 += [(lambda m=m: qk_proj_chunk(m, nx)) for m in range(4)]
                    else:
                        # block 3: q chunks projected here; the k chunks are
                        # deferred into wave 3 (keys 12-15 are only consumed
                        # from jc=12, and wave 3 is otherwise exp-bound)
                        fill0 += [(lambda m=m: qk_proj_chunk(m, nx)) for m in (0, 2)]
                    fill0 += [(lambda t=t: v_proj_chunk(t)) for t in range(4 * nx, 4 * nx + 4)]
                    if n == 0:
                        def load_wout():
                            nc.sync.dma_start(wout_sb[:], WOUT[:])
                        fill0 += [load_wout]
                    half = (len(fill0) + 1) // 2
                    fill_a += fill0[:half]
                    fill_b += fill0[half:]
                pre_a = pre_b = None
                if n == 3:
                    # deferred block-3 k projections: keys 12-15 are only
                    # consumed from jc=12, but must be written well before —
                    # force them at the head of each pair's jc loop.  out_proj
                    # 12/13 runs inside pair 1 once both pairs early-evicted
                    # queries [1536:1792).
                    pre_a = {0: [lambda: qk_proj_chunk(1, 3)]}
                    pre_b = {0: [lambda: qk_proj_chunk(3, 3)],
                             14: [lambda: out_proj_chunk(12), lambda: out_proj_chunk(13)]}
                op_sched = {1: range(0, 4), 2: range(4, 8), 3: range(8, 12)}
                ops = [(lambda t=t: out_proj_chunk(t)) for t in op_sched.get(n, ())]
                attn_ic(0, n, fill_a, even_fillers=ops[:2], pre=pre_a)
                attn_ic(1, n, fill_b, even_fillers=ops[2:], pre=pre_b)
            for tcc in range(14, 16):
                out_proj_chunk(tcc)

            if debug:
                nc.sync.dma_start(DBG_QKT[:], qkT[:])
                nc.sync.dma_start(DBG_V[:], v_sb[:])
                nc.sync.dma_start(DBG_ATT[:], attnT[:])

    nc.compile()
    return nc


_DEINT = list(range(0, DK, 2)) + list(range(1, DK, 2))


def _rope_tables():
    j = np.arange(DK // 2, dtype=np.float64)
    inv_freq = THETA ** (-2.0 * j / DK)
    t = np.arange(T, dtype=np.float64)
    ang = t[None, :] * inv_freq[:, None]          # [32, T]
    ang = np.tile(ang, (4, 1))                    # [128, T]
    return np.cos(ang).astype(np.float32), np.sin(ang).astype(np.float32)


def _psw():
    M = np.zeros((128, 128), dtype=np.float32)
    for p in range(128):
        pm = p % 64
        if pm < 32:
            M[p, p + 32] = -1.0
        else:
            M[p, p - 32] = 1.0
    return np.ascontiguousarray(M.T)


def shard_inputs(x, Wqkv, bqkv, Wout, bout):
    bf16 = ml_dtypes.bfloat16
    x = np.asarray(x, dtype=np.float32)
    Wqkv = np.asarray(Wqkv, dtype=np.float32)
    bqkv = np.asarray(bqkv, dtype=np.float32)
    Wout = np.asarray(Wout, dtype=np.float32)

    cos_t, sin_t = _rope_tables()
    cq = np.ascontiguousarray(cos_t / 8.0).astype(bf16)
    sq = np.ascontiguousarray(sin_t / 8.0).astype(bf16)
    psw = _psw().astype(bf16)
    tri = np.triu(np.ones((128, 128), dtype=np.float32)).astype(bf16)

    def pack(w_t, kc):
        # [kc*128, cols] channel-major -> [128, kc*cols] SBUF-contiguous
        cols = w_t.shape[1]
        return np.ascontiguousarray(
            w_t.reshape(kc, 128, cols).transpose(1, 0, 2).reshape(128, kc * cols)
        ).astype(bf16)

    xt = {}
    for b in range(B):
        # [p, n*4096 + k*512 + t] block-major SBUF layout
        xt[b] = np.ascontiguousarray(
            x[b].T.reshape(KC, 128, TC512, 512).transpose(1, 2, 0, 3).reshape(128, -1)
        ).astype(bf16)

    in_maps = []
    for c in range(NCORES):
        b = c // 4
        heads = [4 * (c % 4) + i for i in range(HEADS_PER_CORE)]
        # chunk order: [Qp0 | Kp0 | Qp1 | Kp1], each 128 rows (2 heads x 64)
        qk_rows = []
        for p in range(2):
            qrows, krows = [], []
            for h in (2 * p, 2 * p + 1):
                H = heads[h]
                qrows += [H * 192 + j for j in _DEINT]
                krows += [H * 192 + 64 + j for j in _DEINT]
            qk_rows += qrows + krows
        v_rows = []
        for h in range(4):
            H = heads[h]
            v_rows += [H * 192 + 128 + j for j in range(DK)]
        vch_out = []
        for h in range(4):
            H = heads[h]
            vch_out += [H * 64 + j for j in range(DK)]

        in_maps.append({
            "XT": xt[b],
            "WQK": pack(Wqkv[qk_rows].T, KC),
            "WV": pack(Wqkv[v_rows].T, KC),
            "WOUT": pack(Wout[:, vch_out].T, 2),
            "PSW": psw,
            "CQ": cq,
            "SQ": sq,
            "BQK": np.ascontiguousarray(bqkv[qk_rows].reshape(4, 128).T.astype(np.float32)),
            "BV": np.ascontiguousarray(bqkv[v_rows].reshape(1, 256).astype(np.float32)),
            "TRI": tri,
        })
    return in_maps


_CACHED = {}


def _get_program(debug=False):
    key = bool(debug)
    if key not in _CACHED:
        _CACHED[key] = build_program(debug=debug)
    return _CACHED[key]


def run_cores(inputs, debug=False, trace=False, tmpdir=None):
    nc = _get_program(debug=debug)
    in_maps = shard_inputs(**inputs)
    res = run_bass_kernel_spmd(
        nc, in_maps, core_ids=list(range(NCORES)), trace=trace, tmpdir=tmpdir,
    )
    return res


def combine(results, bout):
    bout = np.asarray(bout, dtype=np.float32)
    out = np.empty((B, T, D), dtype=np.float32)
    for b in range(B):
        acc = results[4 * b]["OUT"].astype(np.float32)
        for c in range(4 * b + 1, 4 * b + 4):
            acc += results[c]["OUT"].astype(np.float32)
        out[b] = acc + bout[None, :]
    return out


def kernel(x, Wqkv, bqkv, Wout, bout):
    res = run_cores(dict(x=x, Wqkv=Wqkv, bqkv=bqkv, Wout=Wout, bout=bout))
    return combine(res.results, bout)


# revision 21
# speedup vs baseline: 1.0364x; 1.0326x over previous
"""Causal self-attention with rotary embeddings (B=2, T=2048, D=1024, H=16,
d_k=64) on 8 Trainium2 NeuronCores.

Sharding: core c handles batch b = c//4 and 4 heads (c%4)*4..+4 — data
parallel on B, tensor parallel on heads.  Each core computes its heads'
qkv projection, RoPE, causal attention, and a partial output projection
over its 256 attention channels; the host sums the 4 partials per batch.

Layout tricks:
  * all matmul operands are bf16 (fp32 on the PE costs ~2 cycles/row;
    bf16 is 1).  PSUM accumulation stays fp32.
  * q/k channels are de-interleaved host-side (RoPE pair -> half-split
    form) and packed 2 heads per 128-partition tile; scores matmuls are
    row-tiled K=64 pairs.
  * RoPE swap (+/- sign) is a 128x128 permutation matmul on TensorE; the
    cos/sin elementwise work runs on VectorE fused with PSUM eviction,
    and the qkv bias rides along as the per-partition scalar operand of
    scalar_tensor_tensor (v bias: broadcast add at PSUM eviction).
  * softmax skips max-subtraction (scores ~ N(0,1), bounded) and folds the
    denominator into attn@v as an extra ones-column of v; the divide is a
    per-head broadcast-reciprocal multiply at eviction.
  * causal masking is block-granular: scores/exp/attn@v matmuls under the
    block diagonal are truncated to the live query range.
"""

import sys

sys.path.insert(0, "/opt/trn_rl_repo")

import numpy as np
import ml_dtypes

import concourse.bacc as bacc
import concourse.tile as tile
from concourse import mybir
from concourse.bass_utils import run_bass_kernel_spmd

F32 = mybir.dt.float32
BF16 = mybir.dt.bfloat16

B, T, D = 2, 2048, 1024
NH, DK = 16, 64
THETA = 10000.0
NCORES = 8
HEADS_PER_CORE = 4

TC512 = T // 512        # 4   i-chunks of 512
TC128 = T // 128        # 16  t/j-chunks of 128
KC = D // 128           # 8   d_model contraction chunks

ADD = mybir.AluOpType.add
MUL = mybir.AluOpType.mult


def build_program(debug=False):
    nc = bacc.Bacc("TRN2", target_bir_lowering=False, debug=False)

    # all big operands arrive pre-packed in SBUF layout (partition-major,
    # fully contiguous per partition) so DMA moves 4-8KB bursts instead of
    # 1KB strided runs.  XT is additionally 512-token-block-major.
    XT = nc.dram_tensor("XT", [128, TC512 * KC * 512], BF16, kind="ExternalInput").ap()
    WQK = nc.dram_tensor("WQK", [128, KC * 512], BF16, kind="ExternalInput").ap()
    WV = nc.dram_tensor("WV", [128, KC * 256], BF16, kind="ExternalInput").ap()
    WOUT = nc.dram_tensor("WOUT", [128, 2 * D], BF16, kind="ExternalInput").ap()
    PSW = nc.dram_tensor("PSW", [128, 128], BF16, kind="ExternalInput").ap()
    CQ = nc.dram_tensor("CQ", [128, T], BF16, kind="ExternalInput").ap()
    SQ = nc.dram_tensor("SQ", [128, T], BF16, kind="ExternalInput").ap()
    BQK = nc.dram_tensor("BQK", [128, 4], F32, kind="ExternalInput").ap()
    BV = nc.dram_tensor("BV", [1, 256], F32, kind="ExternalInput").ap()
    TRI = nc.dram_tensor("TRI", [128, 128], BF16, kind="ExternalInput").ap()
    OUT = nc.dram_tensor("OUT", [T, D], BF16, kind="ExternalOutput").ap()
    if debug:
        DBG_QKT = nc.dram_tensor("DBG_QKT", [128, 4 * T], BF16, kind="ExternalOutput").ap()
        DBG_V = nc.dram_tensor("DBG_V", [128, TC128 * 260], BF16, kind="ExternalOutput").ap()
        DBG_ATT = nc.dram_tensor("DBG_ATT", [128, 2 * T], BF16, kind="ExternalOutput").ap()

    EXP = mybir.ActivationFunctionType.Exp

    with tile.TileContext(nc) as tc:
        with (
            tc.tile_pool(name="persist", bufs=1) as persist,
            tc.tile_pool(name="p1w", bufs=1) as p1w,
            tc.tile_pool(name="p1t", bufs=3) as p1t,
            tc.tile_pool(name="p2e", bufs=4) as p2e,
            tc.tile_pool(name="p2bc", bufs=2) as p2bc,
            tc.tile_pool(name="p2r", bufs=2) as p2r,
            tc.tile_pool(name="pj", bufs=2, space="PSUM") as pj,
            tc.tile_pool(name="sps", bufs=2, space="PSUM") as sps,
            tc.tile_pool(name="avps", bufs=2, space="PSUM") as avps,
        ):
            # ---- persistent tiles --------------------------------------
            qkT = persist.tile([128, 4 * T], BF16, tag="qkT")       # Qp0 Kp0 Qp1 Kp1
            v_sb = persist.tile([128, TC128 * 260], BF16, tag="v_sb")  # [jc, head, 64+1]
            attnT = persist.tile([128, 2 * T], BF16, tag="attnT")   # c-chunks x t
            wout_sb = persist.tile([128, 2 * D], BF16, tag="wout_sb")
            tri_sb = persist.tile([128, 128], BF16, tag="tri_sb")

            x_sb = p1w.tile([128, KC * T], BF16, tag="x_sb")
            wqk_sb = p1w.tile([128, KC * 512], BF16, tag="wqk_sb")
            wv_sb = p1w.tile([128, KC * 256], BF16, tag="wv_sb")
            psw_sb = p1w.tile([128, 128], BF16, tag="psw_sb")
            cq_sb = p1w.tile([128, T], BF16, tag="cq_sb")
            sq_sb = p1w.tile([128, T], BF16, tag="sq_sb")
            bqk_sb = p1w.tile([128, 4], F32, tag="bqk_sb")
            bv_sb = p1w.tile([1, 256], F32, tag="bv_sb")
            bvbc_sb = p1w.tile([128, 256], F32, tag="bvbc_sb")

            # x column layout: block-major (n*4096 + k*512 + t), identical
            # in DRAM and SBUF, so every load is a fully-contiguous burst.
            def xcol(k, n512, t0):
                return n512 * (KC * 512) + k * 512 + t0

            def load_x_block(n):
                b0 = n * (KC * 512)
                nc.sync.dma_start(x_sb[:, b0:b0 + 2048], XT[:, b0:b0 + 2048])
                nc.sync.dma_start(x_sb[:, b0 + 2048:b0 + 4096], XT[:, b0 + 2048:b0 + 4096])

            # preamble: x block 0 on gpsimd, wqk on sync (k ascending so
            # the accumulation matmuls start as pieces land); small tables
            # on scalar behind the tiny psw/bias loads.
            nc.gpsimd.dma_start(x_sb[:, 0:512], XT[:, 0:512])
            nc.sync.dma_start(wqk_sb[:, 0:512], WQK[:, 0:512])
            nc.gpsimd.dma_start(x_sb[:, 512:1536], XT[:, 512:1536])
            nc.sync.dma_start(wqk_sb[:, 512:1536], WQK[:, 512:1536])
            nc.gpsimd.dma_start(x_sb[:, 1536:2560], XT[:, 1536:2560])
            nc.sync.dma_start(wqk_sb[:, 1536:2560], WQK[:, 1536:2560])
            nc.gpsimd.dma_start(x_sb[:, 2560:4096], XT[:, 2560:4096])
            nc.sync.dma_start(wqk_sb[:, 2560:4096], WQK[:, 2560:4096])
            nc.scalar.dma_start(psw_sb[:], PSW[:])
            nc.scalar.dma_start(bqk_sb[:], BQK[:])
            nc.scalar.dma_start(bv_sb[:], BV[:])
            nc.scalar.dma_start(cq_sb[:], CQ[:])
            nc.scalar.dma_start(sq_sb[:], SQ[:])
            nc.scalar.dma_start(tri_sb[:], TRI[:])
            nc.gpsimd.dma_start(wv_sb[:], WV[:])
            nc.gpsimd.partition_broadcast(bvbc_sb[:], bv_sb[:], channels=128)

            # ones columns of v_aug: one strided memset
            v4 = v_sb[:].rearrange("p (jc h e) -> p jc h e", jc=TC128, h=4)
            nc.vector.memset(v4[:, :, :, 64:65], 1.0)

            # ---------------- building blocks ---------------------------
            def qk_proj_chunk(m, n):
                """project q/k m-chunk (128 channels) for t-chunk n (512),
                apply bias + rope during eviction."""
                nsl = slice(n * 512, (n + 1) * 512)
                ps = pj.tile([128, 512], F32, tag="pj", name=f"psqk_{m}_{n}")
                for k in range(KC):
                    nc.tensor.matmul(
                        ps[:],
                        wqk_sb[:, k * 512 + m * 128:k * 512 + (m + 1) * 128],
                        x_sb[:, xcol(k, n, 0):xcol(k, n, 512)],
                        start=(k == 0), stop=(k == KC - 1),
                    )
                tmp_s = p1t.tile([128, 512], BF16, tag="tmp_s", name=f"tmps_{m}_{n}")
                tmp_c = p1t.tile([128, 512], BF16, tag="tmp_c", name=f"tmpc_{m}_{n}")
                bcol = bqk_sb[:, m:m + 1]
                nc.vector.scalar_tensor_tensor(tmp_s[:], ps[:], bcol, sq_sb[:, nsl], ADD, MUL)
                nc.vector.scalar_tensor_tensor(tmp_c[:], ps[:], bcol, cq_sb[:, nsl], ADD, MUL)
                sw = pj.tile([128, 512], F32, tag="pj", name=f"sw_{m}_{n}")
                nc.tensor.matmul(sw[:], psw_sb[:], tmp_s[:], start=True, stop=True)
                nc.vector.tensor_add(qkT[:, m * T + n * 512:m * T + (n + 1) * 512], sw[:], tmp_c[:])

            def v_proj_chunk(tcc):
                tsl = slice(tcc * 128, (tcc + 1) * 128)
                psv = pj.tile([128, 256], F32, tag="pj", name=f"psv_{tcc}")
                for k in range(KC):
                    t0 = (tcc % 4) * 128
                    nc.tensor.matmul(
                        psv[:],
                        x_sb[:, xcol(k, tcc // 4, t0):xcol(k, tcc // 4, t0 + 128)],
                        wv_sb[:, k * 256:(k + 1) * 256],
                        start=(k == 0), stop=(k == KC - 1),
                    )
                vdst = v_sb[:, tcc * 260:(tcc + 1) * 260].rearrange(
                    "p (h e) -> p h e", h=4)[:, :, 0:64]
                vsrc = psv[:].rearrange("p (h e) -> p h e", e=64)
                bv4 = bvbc_sb[:].rearrange("p (h e) -> p h e", e=64)
                nc.vector.tensor_add(vdst, vsrc, bv4)

            def evict_heads(p, ic, av, q0, q1):
                """normalize av[q0:q1) into attnT (den reciprocal broadcast)."""
                w = q1 - q0
                for hh in range(2):
                    head = 2 * p + hh
                    den = p2r.tile([1, 512], F32, tag="den", name=f"den_{p}_{ic}_{hh}_{q0}")
                    nc.vector.tensor_copy(den[:, 0:w], av[hh][64:65, q0:q1])
                    rec = p2r.tile([1, 512], F32, tag="rec", name=f"rec_{p}_{ic}_{hh}_{q0}")
                    nc.vector.reciprocal_approx_fast(rec[:, 0:w], den[:, 0:w])
                    bc_sb = p2bc.tile([64, 512], F32, tag="bc_sb", name=f"bc_{p}_{ic}_{hh}_{q0}")
                    nc.gpsimd.partition_broadcast(bc_sb[:, 0:w], rec[:, 0:w], channels=64)
                    cof = (head // 2) * T
                    pof = (head % 2) * 64
                    nc.vector.tensor_mul(
                        attnT[pof:pof + 64, cof + ic * 512 + q0:cof + ic * 512 + q1],
                        av[hh][0:64, q0:q1], bc_sb[:, 0:w],
                    )

            def attn_ic(p, ic, fillers=(), even_fillers=(), pre=None):
                """attention for head-pair p, query chunk ic (512 queries).
                fillers: DMA-gated work, paced late (emitted only when the
                remaining jc slots require it).  even_fillers: compute-ready
                work paced evenly across the jc loop.  pre: {jc: [fns]}
                forced placements.  Queries [0:256) are evicted early at
                jc=njc-3 (their causal blocks are complete), shortening the
                tail chain and freeing av for the next attn_ic sooner."""
                fillers = list(fillers)
                even = list(even_fillers)
                pre = dict(pre or {})
                total, done = len(fillers), 0
                etotal, edone = len(even), 0
                qof = (2 * p) * T
                kof = (2 * p + 1) * T
                njc = 4 * ic + 4
                av = [avps.tile([65, 512], F32, tag="av", name=f"av_{p}_{ic}_{i}") for i in range(2)]
                for jc in range(njc):
                    for fn in pre.pop(jc, ()):
                        fn()
                    rel = jc - 4 * ic
                    ls = 0 if rel < 0 else rel * 128
                    e_pair = p2e.tile([128, 1024], BF16, tag="e_t", name=f"e_{p}_{ic}_{jc}")
                    s_pair = sps.tile([128, 1024], F32, tag="s_ps", name=f"s_{p}_{ic}_{jc}")
                    for hh in range(2):
                        pof = hh * 64
                        nc.tensor.matmul(
                            s_pair[:, hh * 512 + ls:(hh + 1) * 512],
                            qkT[pof:pof + 64, kof + jc * 128:kof + (jc + 1) * 128],
                            qkT[pof:pof + 64, qof + ic * 512 + ls:qof + (ic + 1) * 512],
                            start=True, stop=True,
                        )
                    sv = s_pair[:].rearrange("p (h w) -> p h w", h=2)
                    ev = e_pair[:].rearrange("p (h w) -> p h w", h=2)
                    nc.scalar.activation(ev[:, :, ls:512], sv[:, :, ls:512], EXP, scale=8.0)
                    if rel >= 0:
                        tsl_ = slice(rel * 128, (rel + 1) * 128)
                        nc.vector.tensor_mul(ev[:, :, tsl_], ev[:, :, tsl_],
                                             tri_sb[:].unsqueeze(1).broadcast_to([128, 2, 128]))
                    for hh in range(2):
                        nc.tensor.matmul(
                            av[hh][:, ls:512],
                            v_sb[:, jc * 260 + (2 * p + hh) * 65:jc * 260 + (2 * p + hh) * 65 + 65],
                            e_pair[:, hh * 512 + ls:(hh + 1) * 512],
                            start=(jc == 0), stop=(jc == njc - 1),
                            skip_group_check=True,
                        )
                    if jc == njc - 3:
                        evict_heads(p, ic, av, 0, 256)
                    ewant = -(-etotal * (jc + 1) // njc)
                    while edone < ewant:
                        even[edone]()
                        edone += 1
                    while done < total and (total - done) > (njc - 1 - jc):
                        fillers[done]()
                        done += 1
                evict_heads(p, ic, av, 256, 512)

            def out_proj_chunk(tcc):
                tsl = slice(tcc * 128, (tcc + 1) * 128)
                po_sb = p1t.tile([128, 1024], BF16, tag="po_sb", name=f"po_sb_{tcc}")
                for oc in range(2):
                    po = pj.tile([128, 512], F32, tag="pj", name=f"po_{tcc}_{oc}")
                    for cc in range(2):
                        nc.tensor.matmul(
                            po[:],
                            attnT[:, cc * T + tcc * 128:cc * T + (tcc + 1) * 128],
                            wout_sb[:, cc * D + oc * 512:cc * D + (oc + 1) * 512],
                            start=(cc == 0), stop=(cc == 1),
                        )
                    osl = slice(oc * 512, (oc + 1) * 512)
                    if oc == 0:
                        nc.vector.tensor_copy(po_sb[:, osl], po[:])
                    else:
                        nc.scalar.copy(po_sb[:, osl], po[:])
                nc.sync.dma_start(OUT[tsl, :], po_sb[:])

            # ---------------- schedule: n-major waves -------------------
            # wave n: project all qk m-chunks + v chunks for t-block n, run
            # both pairs' attention for query block n, and the out
            # projection for t-chunks completed in wave n-1.
            load_x_block(1)
            for m in range(4):
                qk_proj_chunk(m, 0)
            for tcc in range(4):
                v_proj_chunk(tcc)
            for n in range(TC512):
                fill_a, fill_b = [], []
                if n < 3:
                    nx = n + 1
                    fill0 = []
                    if nx + 1 < TC512:
                        fill0 += [lambda b=nx + 1: load_x_block(b)]
                    if nx < 3:
                        fill0 += [(lambda m=m: qk_proj_chunk(m, nx)) for m in range(4)]
                    else:
                        # block 3: q chunks projected here; the k chunks are
                        # deferred into wave 3 (keys 12-15 are only consumed
                        # from jc=12, and wave 3 is otherwise exp-bound)
                        fill0 += [(lambda m=m: qk_proj_chunk(m, nx)) for m in (0, 2)]
                    fill0 += [(lambda t=t: v_proj_chunk(t)) for t in range(4 * nx, 4 * nx + 4)]
                    if n == 0:
                        def load_wout():
                            nc.sync.dma_start(wout_sb[:], WOUT[:])
                        fill0 += [load_wout]
                    half = (len(fill0) + 1) // 2
                    fill_a += fill0[:half]
                    fill_b += fill0[half:]
                pre_a = pre_b = None
                if n == 3:
                    # deferred block-3 k projections: keys 12-15 are only
                    # consumed from jc=12, but must be written well before —
                    # force them at the head of each pair's jc loop.  out_proj
                    # 12/13 runs inside pair 1 once both pairs early-evicted
                    # queries [1536:1792).
                    pre_a = {0: [lambda: qk_proj_chunk(1, 3)]}
                    pre_b = {0: [lambda: qk_proj_chunk(3, 3)],
                             14: [lambda: out_proj_chunk(12), lambda: out_proj_chunk(13)]}
                op_sched = {1: range(0, 2), 2: range(2, 6), 3: range(6, 12)}
                ops = [(lambda t=t: out_proj_chunk(t)) for t in op_sched.get(n, ())]
                attn_ic(0, n, fill_a, even_fillers=ops[:2], pre=pre_a)
                attn_ic(1, n, fill_b, even_fillers=ops[2:], pre=pre_b)
            for tcc in range(14, 16):
                out_proj_chunk(tcc)

            if debug:
                nc.sync.dma_start(DBG_QKT[:], qkT[:])
                nc.sync.dma_start(DBG_V[:], v_sb[:])
                nc.sync.dma_start(DBG_ATT[:], attnT[:])

    nc.compile()
    return nc


_DEINT = list(range(0, DK, 2)) + list(range(1, DK, 2))


def _rope_tables():
    j = np.arange(DK // 2, dtype=np.float64)
    inv_freq = THETA ** (-2.0 * j / DK)
    t = np.arange(T, dtype=np.float64)
    ang = t[None, :] * inv_freq[:, None]          # [32, T]
    ang = np.tile(ang, (4, 1))                    # [128, T]
    return np.cos(ang).astype(np.float32), np.sin(ang).astype(np.float32)


def _psw():
    M = np.zeros((128, 128), dtype=np.float32)
    for p in range(128):
        pm = p % 64
        if pm < 32:
            M[p, p + 32] = -1.0
        else:
            M[p, p - 32] = 1.0
    return np.ascontiguousarray(M.T)


def shard_inputs(x, Wqkv, bqkv, Wout, bout):
    bf16 = ml_dtypes.bfloat16
    x = np.asarray(x, dtype=np.float32)
    Wqkv = np.asarray(Wqkv, dtype=np.float32)
    bqkv = np.asarray(bqkv, dtype=np.float32)
    Wout = np.asarray(Wout, dtype=np.float32)

    cos_t, sin_t = _rope_tables()
    cq = np.ascontiguousarray(cos_t / 8.0).astype(bf16)
    sq = np.ascontiguousarray(sin_t / 8.0).astype(bf16)
    psw = _psw().astype(bf16)
    tri = np.triu(np.ones((128, 128), dtype=np.float32)).astype(bf16)

    def pack(w_t, kc):
        # [kc*128, cols] channel-major -> [128, kc*cols] SBUF-contiguous
        cols = w_t.shape[1]
        return np.ascontiguousarray(
            w_t.reshape(kc, 128, cols).transpose(1, 0, 2).reshape(128, kc * cols)
        ).astype(bf16)

    xt = {}
    for b in range(B):
        # [p, n*4096 + k*512 + t] block-major SBUF layout
        xt[b] = np.ascontiguousarray(
            x[b].T.reshape(KC, 128, TC512, 512).transpose(1, 2, 0, 3).reshape(128, -1)
        ).astype(bf16)

    in_maps = []
    for c in range(NCORES):
        b = c // 4
        heads = [4 * (c % 4) + i for i in range(HEADS_PER_CORE)]
        # chunk order: [Qp0 | Kp0 | Qp1 | Kp1], each 128 rows (2 heads x 64)
        qk_rows = []
        for p in range(2):
            qrows, krows = [], []
            for h in (2 * p, 2 * p + 1):
                H = heads[h]
                qrows += [H * 192 + j for j in _DEINT]
                krows += [H * 192 + 64 + j for j in _DEINT]
            qk_rows += qrows + krows
        v_rows = []
        for h in range(4):
            H = heads[h]
            v_rows += [H * 192 + 128 + j for j in range(DK)]
        vch_out = []
        for h in range(4):
            H = heads[h]
            vch_out += [H * 64 + j for j in range(DK)]

        in_maps.append({
            "XT": xt[b],
            "WQK": pack(Wqkv[qk_rows].T, KC),
            "WV": pack(Wqkv[v_rows].T, KC),
            "WOUT": pack(Wout[:, vch_out].T, 2),
            "PSW": psw,
            "CQ": cq,
            "SQ": sq,
            "BQK": np.ascontiguousarray(bqkv[qk_rows].reshape(4, 128).T.astype(np.float32)),
            "BV": np.ascontiguousarray(bqkv[v_rows].reshape(1, 256).astype(np.float32)),
            "TRI": tri,
        })
    return in_maps


_CACHED = {}


def _get_program(debug=False):
    key = bool(debug)
    if key not in _CACHED:
        _CACHED[key] = build_program(debug=debug)
    return _CACHED[key]


def run_cores(inputs, debug=False, trace=False, tmpdir=None):
    nc = _get_program(debug=debug)
    in_maps = shard_inputs(**inputs)
    res = run_bass_kernel_spmd(
        nc, in_maps, core_ids=list(range(NCORES)), trace=trace, tmpdir=tmpdir,
    )
    return res


def combine(results, bout):
    bout = np.asarray(bout, dtype=np.float32)
    out = np.empty((B, T, D), dtype=np.float32)
    for b in range(B):
        acc = results[4 * b]["OUT"].astype(np.float32)
        for c in range(4 * b + 1, 4 * b + 4):
            acc += results[c]["OUT"].astype(np.float32)
        out[b] = acc + bout[None, :]
    return out


def kernel(x, Wqkv, bqkv, Wout, bout):
    res = run_cores(dict(x=x, Wqkv=Wqkv, bqkv=bqkv, Wout=Wout, bout=bout))
    return combine(res.results, bout)
